# revision 45
# baseline (speedup 1.0000x reference)
"""Trainium2 Bass kernel v7 for nn_AtomwiseLinear (histogram_binning).

Multi-engine pipelined histogram (per core, SPMD x8, no collectives):
  host: degree-balanced assignment of NB=2 nodes to each of 65536 columns;
        column cap E_CAP = max balanced load (~68, adaptive). Each edge
        endpoint becomes one byte z = lo + NB*(1-w) (lo = node slot in
        column, w = other endpoint's type bit), padded 255. Columns live
        on PARTITIONS (p = col%128), entries along the free dim. Only 3
        range planes r'_t = #(b >= t), t=1..3, are needed (the 4th bin is
        recovered from a shipped per-column valid count cval); pad bytes
        fold into r' linearly. Dense node order is strip-interleaved,
        j = (q%4)*MCOLS + (q//4)*128 + p, so a block of 8 hist iterations
        maps exactly onto one 8192-column mask chunk of y in all 4 strips.
  device, per slot (32 total, lanes spread evenly in time):
    compares on a per-slot engine lane: ACT (Sign activation on bf16
    stream, per-slot sum fixup), DVE (4x-mode is_gt on bf16 stream), or
    Pool (is_gt on u8 stream); entry sums via tree-halving adds (DVE 2x
    mode or Pool) + one short DVE reduce. Dense y = x@W (bf16) on PE with
    ACT copies into an SBUF-resident y.
  after every 8 slots: decode crit (bf16 DVE ops on a [128, 256] q-slice),
    bounce crit (fp8) to DRAM in dense order, broadcast-load critb, mask
    y in place (DVE/Pool), DMA the chunk out as bf16 — fully overlapped
    with the remaining histogram slots.
  host: inverse-permute into [1M, 30] float32.
"""

import os
import sys

sys.path.insert(0, "/opt/trn_rl_repo")

import numpy as np
import ml_dtypes

from concourse import bacc, bass, mybir
import concourse.tile as tile
from concourse.bass_utils import run_bass_kernel_spmd

BF16 = ml_dtypes.bfloat16

NCORES = 8
N_NODES = 1_000_000
D = 30
WINDOW = 5
DEG_THRESH = 10

NB = 2                    # nodes per column
ZW = 2 * NB               # byte-code range
NCOL = 131072 // NB       # columns per core
NG = NCOL // 128          # column groups (one column per partition)
GB = 16                   # groups per iteration
NIT = NG // GB            # iterations (slots)
Q = NB * NG               # crit cols
SE = NB * NCOL            # nodes (incl ghosts) per core
MCOLS = SE // 4           # dense columns (32-partition strips)
BLK = 8                   # slots per decode/mask block
NBLK = NIT // BLK
CH = MCOLS // NBLK        # mask chunk cols (8192)

F32 = mybir.dt.float32
BF = mybir.dt.bfloat16
U8 = mybir.dt.uint8
FP8 = mybir.dt.float8e4
FP8_NP = mybir.dt.np(FP8)

# --- engine-lane knobs ---
N_ACT = int(os.environ.get("N_ACT", "12"))
N_POOL = int(os.environ.get("N_POOL", "8"))
N_DVE = NIT - N_ACT - N_POOL
R_POOL = int(os.environ.get("R_POOL", "5"))   # slots whose tree-adds run on Pool
MSPLIT = float(os.environ.get("MSPLIT", "0.75"))  # DVE share of mask cols


def _lane_map():
    """Per-slot compare lane ('A'/'V'/'P'), each lane spread evenly."""
    cnt = [("A", N_ACT), ("V", N_DVE), ("P", N_POOL)]
    acc = {k: 0.0 for k, _ in cnt}
    lanes = []
    for _ in range(NIT):
        for k, c in cnt:
            acc[k] += c / NIT
        pick = max(acc, key=lambda k: acc[k])
        acc[pick] -= 1.0
        lanes.append(pick)
    return lanes


LANES = _lane_map()
BSLOTS = [it for it in range(NIT) if LANES[it] in ("A", "V")]
PSLOTS = [it for it in range(NIT) if LANES[it] == "P"]
_ROW = {}
for _i, _it in enumerate(BSLOTS):
    _ROW[_it] = _i
for _i, _it in enumerate(PSLOTS):
    _ROW[_it] = _i


def _pool_tree_slots():
    """Spread R_POOL tree-add slots over the non-Pool-compare slots."""
    cand = [it for it in range(NIT) if LANES[it] == "A"] or list(range(NIT))
    if R_POOL <= 0:
        return set()
    n = min(R_POOL, len(cand))
    step = len(cand) / n
    return {cand[int(i * step)] for i in range(n)}


PTREE = _pool_tree_slots()


def _balance(deg, ncol_g):
    """Assign NB nodes to each of ncol_g global columns, equalizing the
    per-column degree sums (matched greedy per round)."""
    NT = NB * ncol_g
    d = np.zeros(NT, np.int64)
    d[:deg.shape[0]] = deg
    order = np.argsort(-d, kind="stable")
    loads = np.zeros(ncol_g, np.int64)
    col = np.empty(NT, np.int64)
    slot = np.empty(NT, np.int64)
    for r in range(NB):
        seg = order[r * ncol_g:(r + 1) * ncol_g]           # degrees desc
        tgt = np.argsort(-loads, kind="stable")            # loads desc
        col[seg[::-1]] = tgt                               # asc deg -> desc load
        slot[seg] = r
        loads[tgt] += d[seg[::-1]]
    return col, slot, int(loads.max()), NT


def _host_prep(x, W, edge_index, atom_types):
    n = x.shape[0]
    e0 = np.asarray(edge_index[0], dtype=np.int32)
    e1 = np.asarray(edge_index[1], dtype=np.int32)
    t8 = np.asarray(atom_types, dtype=np.uint8)

    deg = np.bincount(e0, minlength=n) + np.bincount(e1, minlength=n)
    col, slot, maxload, NT = _balance(deg, NCORES * NCOL)
    ECAP = (max(maxload, 64) + 3) // 4 * 4
    NCOL_G = NCORES * NCOL

    lo_n = slot.astype(np.uint8)          # node slot within column [0,NB)
    core_n = col // NCOL
    cl_n = col % NCOL                     # local column

    # --- entry streams: byte z = lo + 2*(1-w), pad 255 ---
    nodes = np.concatenate([e0, e1])
    wbit = np.concatenate([t8[e1], t8[e0]])
    gc = col[nodes]                       # global column of each entry
    order2 = np.argsort(gc, kind="stable")
    gcs = gc[order2]
    sn = nodes[order2]
    sw = wbit[order2]
    counts = np.bincount(gc, minlength=NCOL_G)
    assert counts.max() <= ECAP, (counts.max(), ECAP)
    starts = np.zeros(NCOL_G, dtype=np.int64)
    np.cumsum(counts[:-1], out=starts[1:])
    within = np.arange(nodes.shape[0], dtype=np.int64) - starts[gcs]

    # stream [core][it][p][gb*ECAP + e], column cl = (it*GB+gb)*128 + p
    stream = np.full(NCORES * NIT * 128 * GB * ECAP, 255, np.uint8)
    c_ = gcs // NCOL
    cll = gcs % NCOL
    g_ = cll // 128
    p_ = cll % 128
    it_ = g_ // GB
    gb_ = g_ % GB
    idx = ((c_ * NIT + it_) * 128 + p_) * (GB * ECAP) + gb_ * ECAP + within
    stream[idx] = lo_n[sn] + NB * (1 - sw)
    stream = stream.reshape(NCORES, NIT, 128, GB * ECAP)

    # --- node -> hist/dense position ---
    # node (cl, lo): p = cl%128, g = cl//128; crit q = g*NB + lo.
    # dense j strip-interleaved by block: q = 256k + 4u + s ->
    # j = s*MCOLS + k*CH + p*64 + u  (64B-contiguous critd runs per p)
    g_n = cl_n // 128
    p_n = cl_n % 128
    q_n = g_n * NB + lo_n
    k_n = q_n // 256
    s_n = q_n % 4
    u_n = (q_n % 256) // 4
    jg = core_n * SE + s_n * MCOLS + k_n * CH + p_n * 64 + u_n
    inv = np.empty(NT, np.int64)
    inv[jg] = np.arange(NT)

    xfull = np.zeros((NT, D), np.float32)
    xfull[:n] = np.asarray(x, np.float32)
    tfull = np.zeros(NT, np.uint8)
    tfull[:n] = t8
    xg = xfull[inv]                           # dense order
    tg = tfull[inv]
    xt = np.ascontiguousarray(
        xg.reshape(NCORES, SE, D).transpose(0, 2, 1)
    ).astype(BF16)
    # th stays in (p, q) hist order: th[p, q] = type of node (p, q)
    th = np.zeros((NCORES, 128, Q), np.uint8)
    th[core_n, p_n, q_n] = tfull
    th = th.astype(FP8_NP)

    wsc = np.zeros((D, 32), np.float32)
    wsc[:, :D] = np.asarray(W, np.float64) / np.sqrt(D)
    wsc = wsc.astype(BF16)
    d5v = np.arange(128, dtype=np.float32) % 32
    d5 = np.where(d5v < 30, d5v // WINDOW, 99.0).reshape(128, 1).astype(np.float32)

    # per-column valid-entry count (padding bookkeeping): cval[c][p, g]
    # for local column cl = g*128 + p
    cval = counts.reshape(NCORES, NCOL).reshape(NCORES, NG, 128)
    cval = np.ascontiguousarray(cval.transpose(0, 2, 1)).astype(BF16)

    # engine-lane streams: all bf16 (walrus rejects u8 compares on Pool)
    streamb = stream[:, BSLOTS].astype(BF16)
    streamp = stream[:, PSLOTS].astype(BF16)

    sb = np.broadcast_to(
        -(np.arange(3, dtype=np.float32) + 0.5), (128, 3)).copy()

    in_maps = []
    for c in range(NCORES):
        in_maps.append({
            "streamb": streamb[c], "streamp": streamp[c], "xt": xt[c],
            "th": th[c], "wsc": wsc, "d5": d5, "cval": cval[c], "sb": sb,
        })
    return in_maps, inv, ECAP


def build_nc(shape=128):
    ECAP = shape
    Z3 = 3                      # range planes r'_t = #(b >= t), t = 1..3
    NDG = MCOLS // 512 // NIT   # dense chunks (of 512 cols x 4 strips) per it
    NBF = len(BSLOTS)
    nc = bacc.Bacc("TRN2", target_bir_lowering=False, debug=False,
                   num_devices=NCORES)
    streamb_d = nc.dram_tensor("streamb", [NBF, 128, GB * ECAP], BF,
                               kind="ExternalInput")
    streamp_d = nc.dram_tensor("streamp", [len(PSLOTS), 128, GB * ECAP], BF,
                               kind="ExternalInput")
    xt_d = nc.dram_tensor("xt", [D, SE], BF, kind="ExternalInput")
    th_d = nc.dram_tensor("th", [128, Q], FP8, kind="ExternalInput")
    wsc_d = nc.dram_tensor("wsc", [D, 32], BF, kind="ExternalInput")
    d5_d = nc.dram_tensor("d5", [128, 1], F32, kind="ExternalInput")
    cval_d = nc.dram_tensor("cval", [128, NG], BF, kind="ExternalInput")
    sb_d = nc.dram_tensor("sb", [128, 3], F32, kind="ExternalInput")
    outt_d = nc.dram_tensor("outt", [4, D, MCOLS], BF, kind="ExternalOutput")
    critd = nc.dram_tensor("crit_bounce", [1, SE], FP8)
    AL = mybir.AluOpType
    SIGN = mybir.ActivationFunctionType.Sign

    with tile.TileContext(nc) as tc:
        with tc.tile_pool(name="const", bufs=1) as cpool:
            wsc = cpool.tile([D, 32], BF)
            d5 = cpool.tile([128, 1], F32)
            th = cpool.tile([128, Q], FP8)
            cval = cpool.tile([128, NG], BF)
            hist = cpool.tile([128, NG * Z3], BF)

            nc.sync.dma_start(out=wsc[:], in_=wsc_d[:])
            nc.sync.dma_start(out=d5[:], in_=d5_d[:])
            nc.sync.dma_start(out=th[:], in_=th_d[:])
            nc.sync.dma_start(out=cval[:], in_=cval_d[:])

            sb = cpool.tile([128, Z3], F32)
            nc.sync.dma_start(out=sb[:], in_=sb_d[:])
            sbias = [sb[:, z:z + 1] for z in range(Z3)]

            thf = cpool.tile([128, Q], BF)
            nc.scalar.copy(out=thf[:], in_=th[:])
            av = cpool.tile([128, Q], BF)
            cnt = cpool.tile([128, Q], BF)
            ta = cpool.tile([128, Q], BF)
            tb = cpool.tile([128, Q], BF)
            crit = cpool.tile([128, Q], FP8)

            hz = hist[:].rearrange("p (i z b) -> p i z b", z=Z3, b=GB)
            cv4 = cval[:].rearrange("p (i o b) -> p i o b", o=1, b=GB)

            def v4(t):
                # [128, Q] with q=(i*GB+b)*NB+l viewed in (i, l, b) order
                return t[:].rearrange("p (i b l) -> p i l b", b=GB, l=NB)

            wpool = tc.alloc_tile_pool(name="work", bufs=int(os.environ.get("WBUFS", "6")))
            dpool = tc.alloc_tile_pool(name="dpsum", bufs=int(os.environ.get("DBUFS", "4")), space="PSUM")
            mpool = tc.alloc_tile_pool(name="mask", bufs=2)

            def decode_block(k):
                """crit for q in [256k, 256(k+1)) from hist its [8k, 8k+8)."""
                i0, i1 = k * BLK, (k + 1) * BLK
                q0, q1 = k * BLK * GB * NB, (k + 1) * BLK * GB * NB
                r1 = hz[:, i0:i1, 0:1, :]
                r2 = hz[:, i0:i1, 1:2, :]
                r3 = hz[:, i0:i1, 2:3, :]
                V = nc.vector
                V.tensor_scalar(out=v4(av)[:, i0:i1, 0:1, :], in0=r1,
                                scalar1=-1.0, scalar2=float(ECAP),
                                op0=AL.mult, op1=AL.add)
                V.tensor_tensor(out=v4(av)[:, i0:i1, 1:2, :], in0=r1, in1=r2,
                                op=AL.subtract)
                V.tensor_tensor(out=v4(cnt)[:, i0:i1, 0:1, :], in0=r2, in1=r3,
                                op=AL.subtract)
                V.tensor_tensor(out=v4(cnt)[:, i0:i1, 0:1, :],
                                in0=v4(cnt)[:, i0:i1, 0:1, :],
                                in1=v4(av)[:, i0:i1, 0:1, :], op=AL.add)
                V.tensor_tensor(out=v4(cnt)[:, i0:i1, 1:2, :],
                                in0=cv4[:, i0:i1], in1=v4(cnt)[:, i0:i1, 0:1, :],
                                op=AL.subtract)
                avq = av[:, q0:q1]
                cntq = cnt[:, q0:q1]
                taq = ta[:, q0:q1]
                tbq = tb[:, q0:q1]
                thq = thf[:, q0:q1]
                V.tensor_scalar(out=taq, in0=avq, scalar1=0.0,
                                scalar2=None, op0=AL.is_equal)
                V.tensor_scalar(out=tbq, in0=thq, scalar1=-1.0,
                                scalar2=1.0, op0=AL.mult, op1=AL.add)
                V.tensor_tensor(out=taq, in0=taq, in1=tbq, op=AL.mult)
                V.tensor_tensor(out=tbq, in0=avq, in1=cntq, op=AL.is_equal)
                V.scalar_tensor_tensor(out=tbq, in0=tbq, scalar=2.0,
                                       in1=thq, op0=AL.mult, op1=AL.mult)
                V.tensor_tensor(out=taq, in0=taq, in1=tbq, op=AL.add)
                V.tensor_scalar(out=tbq, in0=cntq, scalar1=0.0,
                                scalar2=None, op0=AL.is_gt)
                V.tensor_tensor(out=taq, in0=taq, in1=tbq, op=AL.mult)
                V.tensor_scalar(out=tbq, in0=cntq,
                                scalar1=float(DEG_THRESH) + 0.5,
                                scalar2=None, op0=AL.is_gt)
                # crit stored strip-major: storage col s*64+u holds q=4u+s,
                # so the DRAM bounce below is contiguous on both sides
                V.scalar_tensor_tensor(
                    out=crit[:, q0:q1].rearrange("p (s u) -> p u s", s=4),
                    in0=tbq.rearrange("p (u s) -> p u s", s=4),
                    in1=taq.rearrange("p (u s) -> p u s", s=4),
                    scalar=3.0, op0=AL.mult, op1=AL.add)
                # q = 256k + 4u + s -> j = s*MCOLS + k*CH + p*64 + u
                nc.sync.dma_start(
                    out=critd[0:1, :].rearrange(
                        "o (ss kk p u) -> (o p) ss kk u",
                        ss=4, p=128, u=64)[:, :, k:k + 1, :],
                    in_=crit[:, q0:q1].rearrange("p (ss u) -> p ss u", ss=4),
                )

            def mask_block(k, yk):
                c0 = k * CH
                critb = mpool.tile([128, CH], FP8, tag="critb")
                for s in range(4):
                    nc.sync.dma_start(
                        out=critb[32 * s:32 * s + 32, :],
                        in_=critd[0:1, bass.ds(s * MCOLS + c0, CH)
                                  ].to_broadcast([32, CH]),
                    )
                # mask multiply: DVE fused compare*mult for the first
                # share; for the rest DVE only compares (Pool can't) and
                # Pool does the multiply
                cd = int(CH * MSPLIT) // 512 * 512
                nc.vector.scalar_tensor_tensor(
                    out=yk[:, 0:cd], in0=critb[:, 0:cd], scalar=d5[:],
                    in1=yk[:, 0:cd], op0=AL.is_equal, op1=AL.mult,
                )
                if cd < CH:
                    nc.vector.tensor_scalar(
                        out=critb[:, cd:CH], in0=critb[:, cd:CH],
                        scalar1=d5[:], scalar2=None, op0=AL.is_equal)
                    nc.gpsimd.tensor_tensor(
                        out=yk[:, cd:CH], in0=yk[:, cd:CH],
                        in1=critb[:, cd:CH], op=AL.mult)
                for s in range(4):
                    nc.sync.dma_start(
                        out=outt_d[bass.ds(s, 1), :, c0:c0 + CH],
                        in_=yk[32 * s:32 * s + D, :],
                    )

            # ---- pipelined: histogram slots + per-block decode/mask ----
            # stream rows are DMA'd in consecutive-row pairs per lane tensor
            # to halve descriptor-generation serial cost
            pair_tiles = {}

            def raw_for(it):
                lane = LANES[it]
                src = streamb_d if lane in ("A", "V") else streamp_d
                dt_ = BF
                tag = "rawb" if lane in ("A", "V") else "rawp"
                r = _ROW[it]
                r0 = r - (r % 2)
                key = (tag, r0)
                if key not in pair_tiles:
                    nrows = min(2, src.shape[0] - r0)
                    t = wpool.tile([128, nrows * GB * ECAP], dt_, tag=tag, bufs=4)
                    nc.sync.dma_start(
                        out=t[:].rearrange("p (r e) -> p r e", r=nrows),
                        in_=src[bass.ds(r0, nrows), :, :].rearrange(
                            "r p e -> p r e"),
                    )
                    pair_tiles[key] = t
                t = pair_tiles[key]
                h = r % 2
                return t[:, h * GB * ECAP:(h + 1) * GB * ECAP]

            ycur = [None]
            for it in range(NIT):
                if it % BLK == 0:
                    yblk = mpool.tile([128, CH], BF, tag="y", bufs=2)
                    ycur[0] = yblk
                lane = LANES[it]
                raw = raw_for(it)
                oh = wpool.tile([128, GB * Z3 * ECAP], BF, tag="oh", bufs=6)
                for z in range(Z3):
                    ohz = oh[:, z * GB * ECAP:(z + 1) * GB * ECAP]
                    if lane == "A":
                        # sign(b - z - 0.5) = +1 if b > z else -1
                        nc.scalar.activation(out=ohz, in_=raw, func=SIGN,
                                             bias=sbias[z])
                    elif lane == "V":
                        nc.vector.tensor_scalar(
                            out=ohz, in0=raw, scalar1=float(z),
                            scalar2=None, op0=AL.is_gt)
                    else:
                        nc.gpsimd.tensor_scalar(
                            out=ohz, in0=raw, scalar1=float(z),
                            scalar2=None, op0=AL.is_gt)
                hslice = hist[:, it * GB * Z3:(it + 1) * GB * Z3]
                with nc.allow_low_precision(reason="counts <= 128 exact"):
                    # tree-halve the entry axis (DVE 2x mode or Pool), then
                    # one short DVE reduce (Pool can't reduce free axes)
                    reng = nc.gpsimd if it in PTREE else nc.vector
                    e = ECAP
                    ohv = oh[:].rearrange("p (g e) -> p g e", e=ECAP)
                    while e % 2 == 0 and e > 20:
                        e //= 2
                        reng.tensor_tensor(
                            out=ohv[:, :, 0:e], in0=ohv[:, :, 0:e],
                            in1=ohv[:, :, e:2 * e], op=AL.add)
                    nc.vector.tensor_reduce(
                        out=hslice, in_=ohv[:, :, 0:e],
                        axis=mybir.AxisListType.X, op=AL.add)
                    if lane == "A":
                        # sign-sum fixup: r' = (s + ECAP) / 2
                        nc.vector.tensor_scalar(
                            out=hslice, in0=hslice, scalar1=0.5,
                            scalar2=float(ECAP) / 2.0,
                            op0=AL.mult, op1=AL.add)
                # dense chunks for this iteration (one xt DMA per slot)
                W512 = NDG * 512
                m1 = it * W512
                xtt = wpool.tile([D, 4 * W512], BF, tag="xtt", bufs=3)
                nc.sync.dma_start(
                    out=xtt[:],
                    in_=xt_d[:].rearrange("d (s m) -> d s m", s=4)[
                        :, :, m1:m1 + W512],
                )
                for u in range(NDG):
                    m0 = m1 + u * 512
                    dps = dpool.tile([128, 512], F32, tag="dps")
                    for s in range(4):
                        nc.tensor.matmul(
                            dps[32 * s:32 * s + 32, :],
                            lhsT=wsc[:],
                            rhs=xtt[:, s * W512 + u * 512:
                                    s * W512 + (u + 1) * 512],
                            start=True, stop=True, tile_position=(0, 32 * s),
                        )
                    nc.scalar.copy(
                        out=ycur[0][:, m0 - (it // BLK) * CH:
                                    m0 - (it // BLK) * CH + 512],
                        in_=dps[:])

                if it % BLK == BLK - 1:
                    k = it // BLK
                    decode_block(k)
                    mask_block(k, ycur[0])

            mpool.release()
            dpool.release()
            wpool.release()

    nc.compile()
    return nc


def _assemble(results, inv, dtype):
    # results[c]["outt"]: [4, 30, MCOLS] bf16, row-major dense order
    big = np.concatenate(
        [np.asarray(results[c]["outt"]).reshape(4, D, MCOLS) for c in range(NCORES)],
        axis=0,
    )
    big = big.transpose(0, 2, 1).reshape(-1, D)
    out = np.empty((N_NODES, D), dtype=dtype)
    sel = inv < N_NODES
    out[inv[sel]] = big[sel].astype(dtype)
    return out


def kernel(x, W, edge_index, atom_types):
    x = np.asarray(x)
    in_maps, inv, ecap = _host_prep(x, W, edge_index, atom_types)
    nc = build_nc(shape=ecap)
    res = run_bass_kernel_spmd(nc, in_maps, list(range(NCORES)))
    return _assemble(res.results, inv, np.float32)
